# revision 16
# baseline (speedup 1.0000x reference)
"""NeighborhoodAttention1D kernel for 8 Trainium2 NeuronCores.

Sequence-parallel sharding: each of the 8 cores handles 1024 consecutive
query positions (batch b = core//4, chunk j = core%4), with a 16-token
K/V halo on each side (zero-padded at batch edges; boundary-clamped
windows never read the padding).

Per-core pipeline (all on-chip after the initial loads):
  phase 1: qkv^T = W_qkv^T-style matmuls producing q^T,k^T in
           [feature, token] layout and V in natural [token, feature]
           layout (bias for V folded in via a ones-row matmul).
  phase 2: neighborhood attention per (head, 128-key chunk): scores are
           computed transposed (S^T = K^T.T @ Q^T blocks), exp on the
           scalar engine, multiplicative 0/1 band mask on the vector
           engine, then the masked-exp block is used as the stationary
           operand of two matmuls: attn@V (natural output) and the
           softmax denominator (ones column).  Normalization happens on
           eviction with a per-partition reciprocal.
  phase 3: PE transposes of the [query, 512] attention output feed the
           output projection (bias via ones-row matmul).

Compute is bf16 on the PE (fp32 matmul is 4 cycles/row on trn2; bf16 is
1), accumulation fp32 in PSUM.
"""

import time
from contextlib import ExitStack

import ml_dtypes
import numpy as np

import concourse.bass as bass
import concourse.tile as tile
from concourse import bacc, mybir
from concourse.bass_utils import run_bass_kernel_spmd
from concourse.masks import make_identity

B, L, DIM = 2, 4096, 512
HEADS, KS = 8, 33
HD = DIM // HEADS          # 64
SCALE = HD ** -0.5
NCORES = 8
CHUNK = 1024               # queries per core
HALO = KS // 2             # 16
TOK = CHUNK + 2 * HALO     # 1056 local tokens per core
NQT = CHUNK // 128         # 8 query tiles
NKC = 9                    # key chunks: 8 full + 1 of 32 rows

BF = mybir.dt.bfloat16
F32 = mybir.dt.float32
NPBF = ml_dtypes.bfloat16

_cache = {}


def _block_geom(c):
    """(kw, q0, qn) for key-chunk block c: key rows [128c, 128c+kw),
    query token columns [q0, q0+qn)."""
    if c == 0:
        return 128, 16, 128
    if c == NKC - 1:
        return 32, 1008, 32
    return 128, 128 * c - 16, 160


def _tile_mask_geom(t):
    """Key rows for query tile t: main = chunk t (128 keys), corner =
    first 32 keys of chunk t+1 (only queries 96:128 of the tile reach
    them; the mask zeroes the rest)."""
    return 16 + 128 * t  # first query token column


def _build_bass(dbg=False):
    nc = bacc.Bacc("TRN2", target_bir_lowering=False, debug=False,
                   num_devices=NCORES)

    xT = nc.dram_tensor("xT", [4, 128, TOK], BF, kind="ExternalInput").ap()
    wqkvT = nc.dram_tensor("wqkvT", [4, 128, 3 * DIM], BF,
                           kind="ExternalInput").ap()
    wprojT = nc.dram_tensor("wprojT", [4, 128, DIM], BF,
                            kind="ExternalInput").ap()
    bqk = nc.dram_tensor("bqk", [128, 8], F32, kind="ExternalInput").ap()
    bv = nc.dram_tensor("bv", [1, DIM], BF, kind="ExternalInput").ap()
    bp = nc.dram_tensor("bp", [1, DIM], BF, kind="ExternalInput").ap()
    masks = nc.dram_tensor("masks", [NQT, 160, 128], BF,
                           kind="ExternalInput").ap()
    out = nc.dram_tensor("out", [CHUNK, DIM], F32, kind="ExternalOutput").ap()
    if dbg:
        d_qkT = nc.dram_tensor("d_qkT", [8, 128, TOK], BF,
                               kind="ExternalOutput").ap()
        d_vnat = nc.dram_tensor("d_vnat", [NKC, 128, DIM], BF,
                                kind="ExternalOutput").ap()
        d_ao = nc.dram_tensor("d_ao", [NQT, 128, DIM], BF,
                              kind="ExternalOutput").ap()
        d_pS = nc.dram_tensor("d_pS", [128, 256], F32,
                              kind="ExternalOutput").ap()
        d_msk = nc.dram_tensor("d_msk", [128, 256], F32,
                               kind="ExternalOutput").ap()
        d_po = nc.dram_tensor("d_po", [128, 65], F32,
                              kind="ExternalOutput").ap()

    with tile.TileContext(nc) as tc, ExitStack() as ctx:
        sb = ctx.enter_context(tc.tile_pool(name="sb", bufs=1))
        ps = ctx.enter_context(tc.tile_pool(name="ps", bufs=1, space="PSUM"))

        # ---- static SBUF ----
        xT_sb = [sb.tile([128, TOK], BF, tag=f"xT{i}", name=f"xT{i}") for i in range(4)]
        wq_sb = [sb.tile([128, 3 * DIM], BF, tag=f"wq{i}", name=f"wq{i}") for i in range(4)]
        wp_sb = [sb.tile([128, DIM], BF, tag=f"wp{i}", name=f"wp{i}") for i in range(4)]
        bqk_sb = sb.tile([128, 8], F32, tag="bqk", name="bqk")
        bv_sb = sb.tile([1, DIM], BF, tag="bv", name="bv")
        bp_sb = sb.tile([1, DIM], BF, tag="bp", name="bp")
        maskm_sb = [sb.tile([128, 128], BF, tag=f"maskm{t}", name=f"maskm{t}")
                    for t in range(NQT)]
        maskc_sb = [sb.tile([32, 128], BF, tag=f"maskc{t}", name=f"maskc{t}")
                    for t in range(NQT)]
        ones_row = sb.tile([1, 128], BF, tag="ones_row", name="ones_row")
        ones_col = sb.tile([128, 1], BF, tag="ones_col", name="ones_col")
        ident = sb.tile([128, 128], BF, tag="ident", name="ident")

        for i in range(4):
            nc.sync.dma_start(xT_sb[i][:], xT[i])
            nc.sync.dma_start(wq_sb[i][:], wqkvT[i])
            nc.sync.dma_start(wp_sb[i][:], wprojT[i])
        nc.sync.dma_start(bqk_sb[:], bqk[:])
        nc.sync.dma_start(bv_sb[:], bv[:])
        nc.sync.dma_start(bp_sb[:], bp[:])
        for t in range(NQT):
            nc.sync.dma_start(maskm_sb[t][:], masks[t, 0:128, :])
            nc.sync.dma_start(maskc_sb[t][:], masks[t, 128:160, :])
        nc.vector.memset(ones_row[:], 1.0)
        nc.vector.memset(ones_col[:], 1.0)
        make_identity(nc, ident[:])

        qkT_sb = [sb.tile([128, TOK], BF, tag=f"qkT{oc}", name=f"qkT{oc}") for oc in range(8)]
        vnat_sb = [sb.tile([128, DIM], BF, tag=f"vnat{t}", name=f"vnat{t}") for t in range(NKC)]
        ao_sb = [sb.tile([128, DIM], BF, tag=f"ao{t}", name=f"ao{t}") for t in range(NQT)]

        work = ctx.enter_context(tc.tile_pool(name="work", bufs=1))

        # ---- phase 1: q^T / k^T ([feature, token]) ----
        TT = [(0, 512), (512, 512), (1024, TOK - 1024)]
        # head h uses q chunk h//2 and k chunk 4+h//2; emit in an order that
        # unblocks head 0 earliest.
        oc_order = [0, 4, 1, 5, 2, 6, 3, 7]

        def emit_qk(oc):
            pt = [ps.tile([128, 512], F32, tag="qkv", name="qkv", bufs=2) for _ in TT]
            for ic in range(4):
                for tt, (t0, tw) in enumerate(TT):
                    nc.tensor.matmul(
                        pt[tt][:, :tw],
                        lhsT=wq_sb[ic][:, oc * 128:(oc + 1) * 128],
                        rhs=xT_sb[ic][:, t0:t0 + tw],
                        start=(ic == 0), stop=(ic == 3),
                    )
            for tt, (t0, tw) in enumerate(TT):
                nc.scalar.activation(
                    out=qkT_sb[oc][:, t0:t0 + tw], in_=pt[tt][:, :tw],
                    func=mybir.ActivationFunctionType.Identity,
                    bias=bqk_sb[:, oc:oc + 1], scale=1.0,
                )

        def emit_vnat(vt):
            pw = 128 if vt < NKC - 1 else TOK - 128 * (NKC - 1)
            p = ps.tile([128, 512], F32, tag="qkv", name="qkv", bufs=2)
            for ic in range(4):
                nc.tensor.matmul(
                    p[:pw, :],
                    lhsT=xT_sb[ic][:, vt * 128:vt * 128 + pw],
                    rhs=wq_sb[ic][:, 2 * DIM:3 * DIM],
                    start=(ic == 0), stop=False,
                )
            nc.tensor.matmul(
                p[:pw, :], lhsT=ones_row[:1, :pw], rhs=bv_sb[:1, :],
                start=False, stop=True,
            )
            nc.vector.tensor_copy(vnat_sb[vt][:pw, :], p[:pw, :])

        emit_qk(0)
        emit_qk(4)
        for vt in range(NKC):
            emit_vnat(vt)
        for oc in oc_order[2:]:
            emit_qk(oc)

        # ---- phase 2: attention ----
        for h in range(HEADS):
            qT = qkT_sb[h // 2][(h % 2) * 64:(h % 2) * 64 + 64, :]
            kT = qkT_sb[4 + h // 2][(h % 2) * 64:(h % 2) * 64 + 64, :]
            for t in range(NQT):
                q0 = 16 + 128 * t
                k0 = 128 * t
                pS = ps.tile([128, 256], F32, tag="S", name="S", bufs=2)
                nc.tensor.matmul(
                    pS[:, 0:128], lhsT=kT[:, k0:k0 + 128],
                    rhs=qT[:, q0:q0 + 128], start=True, stop=True,
                )
                nc.tensor.matmul(
                    pS[0:32, 128:256], lhsT=kT[:, k0 + 128:k0 + 160],
                    rhs=qT[:, q0:q0 + 128], start=True, stop=True,
                    skip_group_check=True,
                )
                msk = work.tile([128, 256], BF, tag="msk", name="msk", bufs=3)
                expS = work.tile([128, 256], BF, tag="expS", name="expS",
                                 bufs=3)
                nc.scalar.activation(out=expS[:, 0:128], in_=pS[:, 0:128],
                                     func=mybir.ActivationFunctionType.Exp)
                nc.scalar.activation(out=expS[0:32, 128:256],
                                     in_=pS[0:32, 128:256],
                                     func=mybir.ActivationFunctionType.Exp)
                nc.vector.tensor_mul(msk[:, 0:128], expS[:, 0:128],
                                     maskm_sb[t][:])
                nc.vector.tensor_mul(msk[0:32, 128:256], expS[0:32, 128:256],
                                     maskc_sb[t][:])

                po = ps.tile([128, 65], F32, tag="po", name="po", bufs=2)
                nc.tensor.matmul(
                    po[:, 0:64], lhsT=msk[:, 0:128],
                    rhs=vnat_sb[t][:, h * 64:h * 64 + 64],
                    start=True, stop=False,
                )
                # NOTE: start=True clears has_written for the whole PSUM
                # bank, so only the first matmul into this tile may set it;
                # later writes to untouched elements initialize them anyway.
                nc.tensor.matmul(
                    po[:, 64:65], lhsT=msk[:, 0:128], rhs=ones_col[:, :1],
                    start=False, stop=False, skip_group_check=True,
                )
                nc.tensor.matmul(
                    po[:, 0:64], lhsT=msk[0:32, 128:256],
                    rhs=vnat_sb[t + 1][0:32, h * 64:h * 64 + 64],
                    start=False, stop=True, skip_group_check=True,
                )
                nc.tensor.matmul(
                    po[:, 64:65], lhsT=msk[0:32, 128:256],
                    rhs=ones_col[0:32, :1],
                    start=False, stop=True, skip_group_check=True,
                )
                r = work.tile([128, 1], F32, tag="r", name="r", bufs=3)
                nc.vector.reciprocal(r[:], po[:, 64:65])
                nc.vector.tensor_scalar_mul(
                    ao_sb[t][:, h * 64:h * 64 + 64], po[:, 0:64], r[:]
                )
                if dbg and h == 0 and t == 1:
                    stg1 = sb.tile([128, 256], F32, name="stg1")
                    nc.vector.tensor_copy(stg1[:], pS[:])
                    nc.sync.dma_start(d_pS[:], stg1[:])
                    stg2 = sb.tile([128, 256], F32, name="stg2")
                    nc.vector.tensor_copy(stg2[:], msk[:])
                    nc.sync.dma_start(d_msk[:], stg2[:])
                    stg3 = sb.tile([128, 65], F32, name="stg3")
                    nc.vector.tensor_copy(stg3[:], po[:])
                    nc.sync.dma_start(d_po[:], stg3[:])

        # ---- phase 3: output projection ----
        for t in range(NQT):
            aoT = []
            for icc in range(4):
                pT = ps.tile([128, 128], BF, tag="pT", name="pT", bufs=1)
                nc.tensor.transpose(
                    pT[:], ao_sb[t][:, icc * 128:(icc + 1) * 128], ident[:]
                )
                aT = work.tile([128, 128], BF, tag="aoT", name="aoT", bufs=8)
                nc.scalar.copy(aT[:], pT[:])
                aoT.append(aT)
            pout = ps.tile([128, 512], F32, tag="pout", name="pout", bufs=1)
            for icc in range(4):
                nc.tensor.matmul(pout[:], lhsT=aoT[icc][:], rhs=wp_sb[icc][:],
                                 start=(icc == 0), stop=False)
            nc.tensor.matmul(pout[:], lhsT=ones_row[:1, :128], rhs=bp_sb[:1, :],
                             start=False, stop=True)
            osb = work.tile([128, 512], F32, tag="osb", name="osb", bufs=3)
            nc.scalar.copy(osb[:], pout[:])
            nc.sync.dma_start(out[t * 128:(t + 1) * 128, :], osb[:])

        if dbg:
            for oc in range(8):
                nc.sync.dma_start(d_qkT[oc], qkT_sb[oc][:])
            for vt in range(NKC):
                nc.sync.dma_start(d_vnat[vt], vnat_sb[vt][:])
            for t in range(NQT):
                nc.sync.dma_start(d_ao[t], ao_sb[t][:])

    nc.finalize()
    return nc


def _host_prep(x, w_qkv, b_qkv, w_proj, b_proj):
    """Build the 8 per-core input maps."""
    x = np.asarray(x, np.float32)
    w_qkv = np.asarray(w_qkv, np.float32)
    b_qkv = np.asarray(b_qkv, np.float32)
    w_proj = np.asarray(w_proj, np.float32)
    b_proj = np.asarray(b_proj, np.float32)

    wt = w_qkv.T.copy()                      # [512, 1536]
    wt[:, :DIM] *= SCALE                     # fold attention scale into W_q
    bq = b_qkv.copy()
    bq[:DIM] *= SCALE
    wqkvT = wt.reshape(DIM, 3 * DIM).astype(NPBF).reshape(4, 128, 3 * DIM)
    wprojT = w_proj.T.copy().astype(NPBF).reshape(4, 128, DIM)
    bqk = bq[:1024].reshape(8, 128).T.copy().astype(np.float32)
    bv = bq[2 * DIM:3 * DIM].reshape(1, DIM).astype(NPBF)
    bpj = b_proj.reshape(1, DIM).astype(NPBF)

    starts = np.clip(np.arange(L) - HALO, 0, L - KS)   # global window starts

    in_maps = []
    for core in range(NCORES):
        b, j = divmod(core, 4)
        base = j * CHUNK - HALO
        lo, hi = max(0, base), min(L, base + TOK)
        xs = np.zeros((TOK, DIM), np.float32)
        xs[lo - base:hi - base] = x[b, lo:hi]
        xTc = xs.T.copy().astype(NPBF).reshape(4, 128, TOK)

        mk = np.zeros((NQT, 160, 128), np.float32)
        for t in range(NQT):
            kg = base + 128 * t + np.arange(160)      # main 0:128, corner 128:160
            qg = base + 16 + 128 * t + np.arange(128)
            ws = starts[qg]
            mk[t] = ((kg[:, None] >= ws[None, :])
                     & (kg[:, None] <= ws[None, :] + KS - 1))
        in_maps.append({
            "xT": xTc, "wqkvT": wqkvT, "wprojT": wprojT,
            "bqk": bqk, "bv": bv, "bp": bpj,
            "masks": mk.astype(NPBF),
        })
    return in_maps


def kernel(x, w_qkv, b_qkv, w_proj, b_proj):
    if "nc" not in _cache:
        _cache["nc"] = _build_bass()
    nc = _cache["nc"]
    in_maps = _host_prep(x, w_qkv, b_qkv, w_proj, b_proj)
    res = run_bass_kernel_spmd(nc, in_maps, core_ids=list(range(NCORES)))
    full = np.empty((B, L, DIM), np.float32)
    for core in range(NCORES):
        b, j = divmod(core, 4)
        full[b, j * CHUNK:(j + 1) * CHUNK] = res.results[core]["out"]
    return full
